# revision 36
# baseline (speedup 1.0000x reference)
"""Trainium2 Bass kernel for nn_Encoder (sliding-window MLP + synaptic conv).

Computation (per timestep t of T_data):
  syn_e[t] = sum(S_e[t, :]);  syn_i[t] = sum(S_i[t, :])
  syn_out[t, s] = sum_k e_kern[s, k] * syn_e[t-k] + i_kern[s, k] * syn_i[t-k]
  Vw[t, :] = V[t-199 : t+200]   (zero padded)
  h = lrelu(Vw @ W1.T + b1); h = lrelu(h @ W2.T + b2); h = lrelu(h @ W3.T + b3)
  out[t, :] = tanh(h @ W4.T + b4 + syn_out[t, :])

Strategy: data-parallel over T across 8 NeuronCores (T/8 slice + 199-halo
per core).  On each core:
  - S_e/S_i are uploaded TRANSPOSED in fp8-e4m3 (row-sum washes out the
    quantization; fp8 quarters the dominant HBM stream).  Row-sums run on
    the PE as ones-stationary DoubleRow matmuls (VectorE reduce has no
    fast uop and would cost ~110us); sums are evacuated in bf16 to a DRAM
    scratch and re-read as Hankel conv operands.
  - The MLP runs in fp8 DoubleRow (2 contraction rows per pass): weights
    are K-padded to 512 and packed [128, 4, 512]; the V window streams
    straight out of the Hankel SBUF tile via an overlapping 3-D AP.
  - Biases ride inside the stationaries: h carries a constant-1 row at
    hid-index 500 (created by L1's evacuation bias), and W2/W3/W4 row 500
    holds b2/b3/b4 (plus a 1.0 diagonal to regenerate the ones row).
  - L4 and the conv accumulate into the SAME [20, nt] PSUM slot (conv
    matmuls join two pairs later, once the sum scratch exists); slots are
    packed 4-per-bank at partitions 0/32/64/96.  A single Tanh evacuates.
  - PSUM->SBUF lrelu evacuations are split between ScalarE (Activation
    with bias) and VectorE (single fused scalar_tensor_tensor max(x,.01x)).
"""

import os
from contextlib import ExitStack

import ml_dtypes
import numpy as np

import concourse.bass as bass
import concourse.mybir as mybir
import concourse.tile as tile
from concourse import bacc
from concourse.bass_utils import run_bass_kernel_spmd
from concourse.tile_rust import add_dep_helper

FP8 = ml_dtypes.float8_e4m3fn
BF16 = ml_dtypes.bfloat16
FP32 = mybir.dt.float32
BF = mybir.dt.bfloat16
F8 = mybir.dt.float8e4
DR = mybir.MatmulPerfMode.DoubleRow

T_NO = 200
WIN = 2 * T_NO - 1  # 399
N_CORES = 8
BLK = 512
HIDP = 512  # hid (500) padded; row 500 = constant-1 / bias row
WINP = 512  # window (399) padded
SUBP = 32   # sub (20) padded in the W4 stationary free dim
VH_W = 384  # extra hankel cols: DR pass1 reads cols up to 256+128+nt-1
SY_W = 72   # conv hankel extra cols (truncated 128-tap kernels, offset 72)

LAST = {}


def _ceil_div(a, b):
    return -(-a // b)


def _build(T_PAD, L_PAD, SUB):
    NB = _ceil_div(T_PAD, BLK)
    SE_G = 8  # 8 tiles of 256 transposed S_e rows (2048 pad)
    SI_G = 2  # 2 tiles of 256 transposed S_i rows (512 pad)
    V_LEN = T_PAD + WIN - 1 + 128
    RB = L_PAD // BLK  # rowsum blocks over the scratch domain

    nc = bacc.Bacc(
        "TRN2", target_bir_lowering=False, debug=False, num_devices=N_CORES
    )

    set_h = nc.dram_tensor("set", [SE_G * 256, L_PAD], F8, kind="ExternalInput")
    sit_h = nc.dram_tensor("sit", [SI_G * 256, L_PAD], F8, kind="ExternalInput")
    v_h = nc.dram_tensor("v", [V_LEN], F8, kind="ExternalInput")
    cp8_h = nc.dram_tensor("cp8", [128, 3 * 4 * HIDP + 4 * SUBP + 2 * 32], F8,
                           kind="ExternalInput")
    cp16_h = nc.dram_tensor("cp16", [128, 2 * SUBP], BF, kind="ExternalInput")
    cpf_h = nc.dram_tensor("cpf", [128, 4], FP32, kind="ExternalInput")
    out_h = nc.dram_tensor("out", [SUB, T_PAD], FP32, kind="ExternalOutput")
    scr_h = nc.dram_tensor("scr", [2, L_PAD], BF)

    with tile.TileContext(nc) as tc, ExitStack() as ctx:
        cpool = ctx.enter_context(tc.tile_pool(name="consts", bufs=1))
        spool = ctx.enter_context(tc.tile_pool(name="sdata", bufs=1))
        tmppool = ctx.enter_context(tc.tile_pool(name="evtmp", bufs=2))
        hkpool = ctx.enter_context(tc.tile_pool(name="hankel", bufs=2))
        sypool = ctx.enter_context(tc.tile_pool(name="synh", bufs=2))
        hpool = ctx.enter_context(tc.tile_pool(name="acts", bufs=2))
        smpool = ctx.enter_context(tc.tile_pool(name="sums", bufs=3))
        opool = ctx.enter_context(tc.tile_pool(name="outs", bufs=3))
        psmm = ctx.enter_context(tc.tile_pool(name="psmm", bufs=3, space="PSUM"))
        ps4p = ctx.enter_context(tc.tile_pool(name="ps4p", bufs=1, space="PSUM"))

        # ---- constants ----
        def cload(nm, shape, dram, off, width):
            t = cpool.tile(shape, dram.dtype, name=nm, tag=nm)
            ap_dims = [[dram.shape[1], 128]]
            rem = shape[1:]
            if len(rem) == 2:
                ap_dims += [[rem[1], rem[0]], [1, rem[1]]]
            else:
                ap_dims += [[1, rem[0]]]
            d = nc.sync.dma_start(
                out=t[tuple([slice(None)] * len(shape))],
                in_=bass.AP(dram, off, ap_dims),
            )
            return t, d

        # startup-critical first: pair 0's hankel, L1's stationary and bias
        vh0 = hkpool.tile([128, 2 * BLK + VH_W], F8, name="vh", tag="vh")
        tot0 = min(T_PAD, 2 * BLK)
        vh0_dma = nc.sync.dma_start(
            out=vh0[:, : tot0 + VH_W],
            in_=bass.AP(v_h, 0, [[1, 128], [1, tot0 + VH_W]]),
        )
        w1t, w_dma = cload("w1t", [128, 4, HIDP], cp8_h, 0, 4 * HIDP)
        bias1, _ = cload("bias1", [128, 4], cpf_h, 0, 4)
        c001 = cpool.tile([128, 1], FP32, name="c001", tag="c001")
        nc.vector.memset(c001[:, :], 0.01)

        # ---- resident transposed S tiles; loaded in column segments ----
        se_sb = spool.tile([128, 2 * SE_G, L_PAD], F8, name="se", tag="se")
        si_sb = spool.tile([128, 2 * SI_G, L_PAD], F8, name="si", tag="si")
        # column segments: two small ones first so the startup rowsums are
        # not stuck behind multi-MB loads; segment s covers rowsum blocks
        # {0}/{1} then pairs (seg s >= 2 covers rb 2s-2, 2s-1)
        seg_bounds = [(0, BLK), (BLK, 2 * BLK)]
        while seg_bounds[-1][1] < L_PAD:
            c0 = seg_bounds[-1][1]
            seg_bounds.append((c0, min(L_PAD, c0 + 2 * BLK)))
        N_SEG = len(seg_bounds)

        def emit_seg(s, gate=None):
            c0, c1 = seg_bounds[s]
            first = None
            for t, ng, dram in ((se_sb, 2 * SE_G, set_h),
                                (si_sb, 2 * SI_G, sit_h)):
                d = nc.sync.dma_start(
                    out=t[:, :, c0:c1],
                    in_=bass.AP(
                        dram, c0,
                        [[L_PAD, 128], [128 * L_PAD, ng], [1, c1 - c0]],
                    ),
                )
                if first is None:
                    first = d
            if gate is not None:
                add_dep_helper(first.ins, gate.ins, sync=True,
                               reason="startup loads first")
            return first

        # ---- pairs of timestep blocks ----
        pairs = [tuple(b for b in (i, i + 1) if b < NB) for i in range(0, NB, 2)]
        NP = len(pairs)
        pair_blks = []
        for pair in pairs:
            blks = []
            off = 0
            for b in pair:
                nt = min(BLK, T_PAD - BLK * b)
                blks.append((BLK * b, nt, off))
                off += nt
            pair_blks.append((blks, off))

        synh_tiles = {}
        ps4_tiles = {}  # pair idx -> psum tile (L4+conv slots @32/64, rowsums @0)

        def ps4_tile(pi):
            if pi not in ps4_tiles:
                ps4_tiles[pi] = ps4p.tile([128, BLK], FP32, name="ps4",
                                          tag=f"ps4_{pi % 2}")
            return ps4_tiles[pi]

        # ---- rowsum block rb -> scratch cols [BLK*rb, BLK*rb+BLK) ----
        # DoubleRow requires dst partition 0: rowsums use rows 0:16 of the
        # current pair's ps4 bank (L4/conv slots sit at partitions 32/64)
        def emit_rowsum(rb, pi):
            c0 = BLK * rb
            nt = min(BLK, L_PAD - c0)
            ps = ps4_tile(pi)
            for g in range(SE_G):
                nc.tensor.matmul(
                    ps[0:16, :nt], seo[:, :, :],
                    se_sb[:, 2 * g:2 * g + 2, c0:c0 + nt],
                    start=(g == 0), stop=False, perf_mode=DR,
                    skip_group_check=True,
                )
            for g in range(SI_G):
                nc.tensor.matmul(
                    ps[0:16, :nt], sio[:, :, :],
                    si_sb[:, 2 * g:2 * g + 2, c0:c0 + nt],
                    start=False, stop=(g == SI_G - 1), perf_mode=DR,
                    skip_group_check=True,
                )
            sm = smpool.tile([2, BLK], BF, name="sums", tag="sums")
            nc.vector.tensor_copy(sm[:, :nt], ps[0:2, :nt])
            nc.sync.dma_start(
                out=bass.AP(scr_h, c0, [[L_PAD, 2], [1, nt]]), in_=sm[:, :nt]
            )

        def emit_synh(pi):
            t0p = BLK * pairs[pi][0]
            tot = pair_blks[pi][1]
            synh = {}
            for row, nm in ((0, "se"), (1, "si")):
                t = sypool.tile([128, 2 * BLK + SY_W], BF, name=f"{nm}h",
                                tag=f"{nm}h")
                nc.sync.dma_start(
                    out=t[:, : tot + SY_W],
                    in_=bass.AP(scr_h, row * L_PAD + t0p,
                                [[1, 128], [1, tot + SY_W]]),
                )
                synh[nm] = t
            synh_tiles[pi] = synh

        def emit_conv(pi):
            # kernels truncated to 128 taps (tails < 1e-6 of peak): the
            # flipped-kernel rows 72..199 stream scr[t0p + 72 + p + n]
            blks, tot = pair_blks[pi]
            synh = synh_tiles.pop(pi)
            ps4 = ps4_tiles.pop(pi)
            for j, (nm, ko) in enumerate((("se", 0), ("si", 1))):
                for bi, (bt0, nt, coff) in enumerate(blks):
                    bp = 32 + 32 * bi
                    nc.tensor.matmul(
                        ps4[bp:bp + SUB, :nt], kpk[:, ko, :SUB],
                        synh[nm][:, coff + 72: coff + 72 + nt],
                        start=False, stop=(j == 1), skip_group_check=True,
                    )
            for bi, (bt0, nt, coff) in enumerate(blks):
                bp = 32 + 32 * bi
                out_sb = opool.tile([SUB, BLK], FP32, name="out_sb", tag="out_sb")
                nc.scalar.activation(out_sb[:, :nt], ps4[bp:bp + SUB, :nt],
                                     mybir.ActivationFunctionType.Tanh)
                nc.sync.dma_start(out=out_h[:, bt0:bt0 + nt], in_=out_sb[:, :nt])

        # evacuation engine per (layer idx 0..2, m-chunk 0..3), pair-merged:
        # one op covers both blocks of the pair via the 2-bank psum tile.
        # L1 needs the bias (ones-row creation) -> Activation only.
        # "V": DVE copies PSUM->SBUF bf16, then applies lrelu SBUF->SBUF
        # (a single DVE op cannot read two PSUM operands; GpSimd has no
        # PSUM port and no TensorScalarPtr opcode).
        EVAC = {
            (0, 0): "A", (0, 1): "A", (0, 2): "A", (0, 3): "A",
            (1, 0): "A", (1, 1): "V", (1, 2): "A", (1, 3): "A",
            (2, 0): "A", (2, 1): "V", (2, 2): "A", (2, 3): "A",
        }

        def emit_evac(lidx, mc, h_t, ps, nbi, nt_last):
            # ps is [128, 2, BLK]; nbi blocks, the last one nt_last wide
            if nbi == 2 and nt_last == BLK:
                src, dst = ps[:, :, :], h_t[:, mc, :, :]
            else:
                src, dst = ps[:, 0:nbi, :nt_last], h_t[:, mc, 0:nbi, :nt_last]
            if EVAC[(lidx, mc)] == "A":
                nc.scalar.activation(
                    dst, src, mybir.ActivationFunctionType.Lrelu,
                    bias=bias1[:, mc:mc + 1] if lidx == 0 else 0.0,
                    alpha=0.01,
                )
            else:
                tmp = tmppool.tile([128, 2, BLK], BF, name="evtmp", tag="evtmp")
                tsrc = tmp[:, 0:nbi, :nt_last]
                nc.vector.tensor_copy(tsrc, src)
                nc.vector.scalar_tensor_tensor(
                    dst, tsrc, c001[:, 0:1], tsrc,
                    mybir.AluOpType.mult, mybir.AluOpType.max,
                )

        rb_next = 0

        def emit_rowsums_until(tgt, pi):
            nonlocal rb_next
            while rb_next < min(tgt, RB):
                emit_rowsum(rb_next, pi)
                rb_next += 1

        vh_tiles = {}

        def emit_vh(pi):
            tot = pair_blks[pi][1]
            t = hkpool.tile([128, 2 * BLK + VH_W], F8, name="vh", tag="vh")
            d = nc.sync.dma_start(
                out=t[:, : tot + VH_W],
                in_=bass.AP(v_h, BLK * pairs[pi][0],
                            [[1, 128], [1, tot + VH_W]]),
            )
            vh_tiles[pi] = t
            return d

        for pi, pair in enumerate(pairs):
            blks, tot = pair_blks[pi]
            nbi = len(blks)
            nt_last = blks[-1][1]

            if pi == 0:
                vh_tiles[0] = vh0
                # the remaining constants load behind the startup pair
                w2t, _ = cload("w2t", [128, 4, HIDP], cp8_h, 4 * HIDP,
                               4 * HIDP)
                w3t, _ = cload("w3t", [128, 4, HIDP], cp8_h, 8 * HIDP,
                               4 * HIDP)
                w4t, _ = cload("w4t", [128, 4, SUBP], cp8_h, 12 * HIDP,
                               4 * SUBP)
                seo, _ = cload("seo", [128, 2, 16], cp8_h,
                               12 * HIDP + 4 * SUBP, 32)
                sio, _ = cload("sio", [128, 2, 16], cp8_h,
                               12 * HIDP + 4 * SUBP + 32, 32)
                kpk, _ = cload("kpk", [128, 2, SUBP], cp16_h, 0, 2 * SUBP)
                for s in range(4):
                    emit_seg(s, gate=w_dma if s == 0 else None)
            vh = vh_tiles.pop(pi)
            if pi >= 1:
                emit_synh(pi - 1)
            if pi == NP - 1:
                emit_synh(pi)
            if pi + 1 < NP:
                emit_vh(pi + 1)  # prefetch next pair's hankel
            if pi + 4 < N_SEG:
                emit_seg(pi + 4)
            ps4_tile(pi)

            # layers 1..3, fp8 DoubleRow; loops ordered (mc, P, block) so
            # consecutive matmuls share the stationary; rowsum and conv
            # matmuls slot in at layer boundaries to keep the PE fed while
            # the evacuations drain
            h_prev = None
            for lidx, w_t in enumerate((w1t, w2t, w3t)):
                h_t = hpool.tile([128, 4, 2, BLK], F8, name=f"h{lidx}",
                                 tag=f"h{lidx}")
                for mc in range(4):
                    ps = psmm.tile([128, 2, BLK], FP32, name="ps", tag="ps")
                    for P in range(2):
                        for bi, (bt0, nt, coff) in enumerate(blks):
                            if lidx == 0:
                                vb = vh[:, :]
                                rhs = bass.AP(
                                    vb.tensor, vb.offset + coff + 256 * P,
                                    [[2 * BLK + VH_W, 128], [128, 2], [1, nt]],
                                )
                            else:
                                rhs = h_prev[:, 2 * P:2 * P + 2, bi, :nt]
                            nc.tensor.matmul(
                                ps[:, bi, :nt],
                                w_t[:, 2 * P:2 * P + 2,
                                    128 * mc:128 * (mc + 1)],
                                rhs,
                                start=(P == 0), stop=(P == 1), perf_mode=DR,
                                skip_group_check=True,
                            )
                    emit_evac(lidx, mc, h_t, ps, nbi, nt_last)
                h_prev = h_t
                if lidx == 0:
                    if pi >= 1:
                        emit_rowsums_until(2 * pi + 2, pi)
                elif lidx == 1:
                    emit_rowsums_until(2 * pi + 3 if pi else 2, pi)
                    if pi >= 1:
                        emit_conv(pi - 1)
                elif pi == 0:
                    emit_rowsums_until(3, pi)

            # layer 4 feed-forward into the shared ps4 slots (conv joins one
            # pair later); plain fp8 matmuls: DoubleRow requires dst
            # partition 0, which the rowsums own
            ps4 = ps4_tile(pi)
            for kc in range(4):
                for bi, (bt0, nt, coff) in enumerate(blks):
                    bp = 32 + 32 * bi
                    nc.tensor.matmul(
                        ps4[bp:bp + SUB, :nt],
                        w4t[:, kc, :SUB],
                        h_prev[:, kc, bi, :nt],
                        start=(kc == 0), stop=False,
                        skip_group_check=True,
                    )
            if pi == NP - 1:
                emit_conv(pi)

    nc.compile()
    return nc


def kernel(V, S_e, S_i, W1, b1, W2, b2, W3, b3, W4, b4, W_syn, Tau_syn, Delta_syn):
    V = np.asarray(V, np.float32)
    T = V.shape[0]
    assert T % N_CORES == 0
    T_LOC = T // N_CORES
    T_PAD = _ceil_div(T_LOC, 128) * 128
    halo = T_NO - 1
    L_PAD = _ceil_div(T_LOC + halo, BLK) * BLK  # transposed-S / scratch cols
    V_LEN = T_PAD + WIN - 1 + 128
    HID = W1.shape[0]
    SUB = W4.shape[0]

    # ---- synaptic kernels (tiny, host fp32) ----
    W_syn = np.asarray(W_syn, np.float32)
    Tau_syn = np.asarray(Tau_syn, np.float32)
    Delta_syn = np.asarray(Delta_syn, np.float32)
    t_raw = np.arange(T_NO, dtype=np.float32)[None, :]
    tt_e = np.maximum(t_raw - Delta_syn[:, 0:1], 0.0) / Tau_syn[:, 0:1] ** 2
    tt_i = np.maximum(t_raw - Delta_syn[:, 1:2], 0.0) / Tau_syn[:, 1:2] ** 2
    e_kern = tt_e * np.exp(-tt_e) * W_syn[:, 0:1] ** 2
    i_kern = -(tt_i * np.exp(-tt_i)) * W_syn[:, 1:2] ** 2
    ekm = np.ascontiguousarray(e_kern[:, ::-1].T)  # [T_NO, SUB]
    ikm = np.ascontiguousarray(i_kern[:, ::-1].T)
    # conv kernels truncated to 128 taps (rows 72..199 of the flipped form)
    kpk = np.zeros((128, 2, SUBP), np.float32)
    kpk[:, 0, :SUB] = ekm[T_NO - 128:]
    kpk[:, 1, :SUB] = ikm[T_NO - 128:]

    # ---- DoubleRow weight packs: [128, 4, M], row 500 carries bias/ones ----
    def dr3(mat_pad):
        k, m = mat_pad.shape
        return np.ascontiguousarray(
            mat_pad.reshape(4, 128, m).transpose(1, 0, 2)
        )

    w1p = np.zeros((WINP, HIDP), np.float32)
    w1p[:WIN, :HID] = np.asarray(W1, np.float32).T
    w2p = np.zeros((HIDP, HIDP), np.float32)
    w2p[:HID, :HID] = np.asarray(W2, np.float32).T
    w2p[HID, :HID] = np.asarray(b2, np.float32)
    w2p[HID, HID] = 1.0
    w3p = np.zeros((HIDP, HIDP), np.float32)
    w3p[:HID, :HID] = np.asarray(W3, np.float32).T
    w3p[HID, :HID] = np.asarray(b3, np.float32)
    w3p[HID, HID] = 1.0
    w4p = np.zeros((HIDP, SUBP), np.float32)
    w4p[:HID, :SUB] = np.asarray(W4, np.float32).T
    w4p[HID, :SUB] = np.asarray(b4, np.float32)

    seo = np.zeros((128, 2, 16), np.float32)
    seo[:, :, 0] = 1.0
    sio = np.zeros((128, 2, 16), np.float32)
    sio[:, :, 1] = 1.0
    cp8 = np.concatenate(
        [dr3(w1p).reshape(128, -1), dr3(w2p).reshape(128, -1),
         dr3(w3p).reshape(128, -1), dr3(w4p).reshape(128, -1),
         seo.reshape(128, -1), sio.reshape(128, -1)], axis=1
    ).astype(FP8)

    bias1 = np.zeros((128, 4), np.float32)
    b1f = np.asarray(b1, np.float32)
    for mc in range(4):
        rows = b1f[128 * mc: min(HID, 128 * (mc + 1))]
        bias1[: rows.shape[0], mc] = rows
    bias1[HID - 384, 3] = 1.0  # ones-row seed in h1

    # ---- transposed fp8 S uploads (padded [2048|512, L_PAD]) ----
    S_e8 = np.asarray(S_e, np.float32).astype(FP8)
    S_i8 = np.asarray(S_i, np.float32).astype(FP8)
    SeT = np.ascontiguousarray(S_e8.T)  # [2000, T]
    SiT = np.ascontiguousarray(S_i8.T)  # [500, T]

    vg = np.zeros(halo + T + WIN + 128 + T_PAD - T_LOC, np.float32)
    vg[halo: halo + T] = V
    vg = vg.astype(FP8)

    wd = {
        "cp8": cp8,
        "cp16": np.ascontiguousarray(kpk.reshape(128, -1)).astype(BF16),
        "cpf": np.ascontiguousarray(bias1),
    }
    in_maps = []
    for m in range(N_CORES):
        r0 = m * T_LOC
        set_m = np.zeros((2048, L_PAD), FP8)
        sit_m = np.zeros((512, L_PAD), FP8)
        if m == 0:
            set_m[:SeT.shape[0], halo: halo + T_LOC] = SeT[:, :T_LOC]
            sit_m[:SiT.shape[0], halo: halo + T_LOC] = SiT[:, :T_LOC]
        else:
            set_m[:SeT.shape[0], : halo + T_LOC] = SeT[:, r0 - halo: r0 + T_LOC]
            sit_m[:SiT.shape[0], : halo + T_LOC] = SiT[:, r0 - halo: r0 + T_LOC]
        in_maps.append(
            {"set": set_m, "sit": sit_m, "v": vg[r0: r0 + V_LEN], **wd}
        )

    nc = _build(T_PAD, L_PAD, SUB)
    trace = os.environ.get("CC_TRACE") == "1"
    res = run_bass_kernel_spmd(nc, in_maps, list(range(N_CORES)), trace=trace)
    LAST["exec_time_ns"] = res.exec_time_ns
    LAST["results"] = res
    out = np.concatenate(
        [res.results[m]["out"][:, :T_LOC].T for m in range(N_CORES)], 0
    )
    return np.ascontiguousarray(out.astype(np.float32))
